# revision 6
# baseline (speedup 1.0000x reference)
"""KVMemory kernel for Trainium2 (8 NeuronCores, Bass/Tile).

Strategy (v13):
  - Data-parallel over batch. The host sorts the 4096 examples by
    pair_length, deals 128-example blocks snake-wise to the 8 cores, and
    tile slot j computes only k_sched[j] <= 50 keys (pair_length averages
    ~25 of 50, so half the key/value positions are softmax-masked; the
    sort turns that waste into real savings). Outputs are inverse-permuted
    on the host.
  - Embedding rows are fetched with the ANT dma_gather instruction in
    1024-row chunks (the SWDGE descriptor ring holds 1024 descriptors; a
    single larger gather wedges the Q7 ucode -- found the hard way).
    That is still ~50x fewer SWDGE instructions than per-column indirect
    DMA, whose multi-column offset form returns wrong data on hardware.
    dma_gather's int16 index limit is satisfied by host-side per-core
    table compaction: each core touches <= 25600 < 2^15 unique rows per
    table, so the host dedups (np.unique) and ships a compact fp16 table
    plus pre-wrapped int16 index tensors.
  - Three-engine compute split per tile (all ops verified against the
    neuronxcc ISA check and hardware):
      DVE:  broadcast-TT q*kemb product (2x perf mode), tensor_scalar 4x
            rowsums and scales, 2x tree adds, softmax small ops
      ACT:  activation(Copy) rowsums + wsum scales, Exp
      Pool: gather emission, wsum tensor_scalar+tensor_tensor ping-pong
  - Software pipelining: gathers for tile t+1 and the weighted sum of
    tile t-1 interleave with tile t's logits/softmax, ordered so no
    in-order engine queue blocks on unready work.
"""

import sys

if "/opt/trn_rl_repo" not in sys.path:
    sys.path.insert(0, "/opt/trn_rl_repo")

import numpy as np

import concourse.bass as bass
import concourse.mybir as mybir
import concourse.tile as tile
from concourse import bacc
from concourse import library_config
import concourse.bass_utils as bass_utils

N_CORES = 8
B = 4096
K = 50
D = 256
NUM_KEYS = 100000
NUM_VALUES = 100000
BC = B // N_CORES
P = 128
NTILES = BC // P
NBLOCKS = B // P
CAP = BC * K            # compact table rows per core (worst case, < 2^15)
MASK_NEG = np.float32(-1e30)

R_ACT = 0.40    # fraction of rowsums on ACT (rest on DVE)
W_ACT = 0.35    # fraction of wsum scales on ACT (tree on DVE)
W_POOL = 0.0   # fraction of wsum handled by Pool scale+add pairs

_PROGRAM_CACHE = {}


def _splits(kt):
    """Returns (rd, rc, wd, wc, wp): rowsum DVE/ACT counts; wsum DVE/ACT/Pool."""
    rc = int(round(kt * R_ACT))
    rd = kt - rc
    wp = int(round(kt * W_POOL))
    wc = int(round(kt * W_ACT))
    wd = kt - wp - wc
    if wd < 1:
        wd = 1
        wc = max(0, kt - wd - wp)
        wp = kt - wd - wc
    return rd, rc, wd, wc, wp

GCHUNK = 8  # slabs per dma_gather (1024 rows; SWDGE ring holds 1024 descs)


def _build_program(k_sched):
    f16 = mybir.dt.float16
    f32 = mybir.dt.float32
    i16 = mybir.dt.int16
    A = mybir.AluOpType
    KMAX = max(k_sched)
    ksum = sum(k_sched)
    nc = bacc.Bacc("TRN2", target_bir_lowering=False, debug=False, num_devices=N_CORES)

    kidx_d = nc.dram_tensor("kidx", [P, ksum * 8], i16, kind="ExternalInput")
    vidx_d = nc.dram_tensor("vidx", [P, ksum * 8], i16, kind="ExternalInput")
    bias_d = nc.dram_tensor("bias", [BC, K], f32, kind="ExternalInput")
    query_d = nc.dram_tensor("query", [BC, D], f16, kind="ExternalInput")
    ktab_d = nc.dram_tensor("key_table", [CAP, D], f16, kind="ExternalInput")
    vtab_d = nc.dram_tensor("value_table", [CAP, D], f16, kind="ExternalInput")
    out_d = nc.dram_tensor("out", [BC, D], f32, kind="ExternalOutput")

    idx_off = [0]
    for kt in k_sched:
        idx_off.append(idx_off[-1] + kt * 8)

    with tile.TileContext(nc) as tc:
        with (
            tc.tile_pool(name="kpool", bufs=2) as kpool,
            tc.tile_pool(name="vpool", bufs=3) as vpool,
            tc.tile_pool(name="small", bufs=3) as sp,
        ):
            S = {}
            nc.gpsimd.load_library(library_config.mlp)

            def loads(t):
                kt = k_sched[t]
                rows = slice(t * P, (t + 1) * P)
                cols = slice(idx_off[t], idx_off[t + 1])
                kidx_t = sp.tile([P, KMAX * 8], i16, tag="kidx", name=f"kidx_{t}")
                nc.sync.dma_start(out=kidx_t[:, 0:kt * 8], in_=kidx_d[:, cols])
                vidx_t = sp.tile([P, KMAX * 8], i16, tag="vidx", name=f"vidx_{t}")
                nc.sync.dma_start(out=vidx_t[:, 0:kt * 8], in_=vidx_d[:, cols])
                bias_t = sp.tile([P, K], f32, tag="bias", name=f"bias_{t}")
                nc.sync.dma_start(out=bias_t[:, 0:kt], in_=bias_d[rows, 0:kt])
                q_t = sp.tile([P, D], f16, tag="q", name=f"q_{t}")
                nc.sync.dma_start(out=q_t[:], in_=query_d[rows, :])
                S[t] = dict(kidx=kidx_t, vidx=vidx_t, bias=bias_t, q=q_t)

            def _gather_chunks(dst, tab, idx, kt):
                for lo in range(0, kt, GCHUNK):
                    hi = min(lo + GCHUNK, kt)
                    n = P * (hi - lo)
                    nc.gpsimd.dma_gather(
                        dst[:, lo:hi, :], tab[:], idx[:, lo * 8:hi * 8], n, n, D,
                    )

            def gather_k(t):
                st = S[t]
                kt = k_sched[t]
                kemb = kpool.tile([P, KMAX, D], f16, tag="kemb", name=f"kemb_{t}")
                _gather_chunks(kemb, ktab_d, st["kidx"], kt)
                st["kemb"] = kemb

            def gather_v(t):
                st = S[t]
                kt = k_sched[t]
                vemb = vpool.tile([P, KMAX, D], f16, tag="vemb", name=f"vemb_{t}")
                _gather_chunks(vemb, vtab_d, st["vidx"], kt)
                st["vemb"] = vemb

            def act_wsum_copies(t):
                st = S[t]
                rd, rc, wd, wc, wp = _splits(k_sched[t])
                vemb, probs = st["vemb"], st["probs"]
                for k in range(wd, wd + wc):
                    nc.scalar.activation(
                        out=vemb[:, k, :], in_=vemb[:, k, :],
                        func=mybir.ActivationFunctionType.Copy,
                        bias=0.0, scale=probs[:, k:k + 1],
                    )

            def pool_wsum(t):
                st = S[t]
                kt = k_sched[t]
                rd, rc, wd, wc, wp = _splits(kt)
                if wp == 0:
                    st["accP"] = None
                    return
                vemb, probs = st["vemb"], st["probs"]
                base = wd + wc
                accP = sp.tile([P, D], f32, tag="accP", name=f"accP_{t}")
                scrP = sp.tile([P, D], f32, tag="scrP", name=f"scrP_{t}")
                nc.gpsimd.tensor_scalar(
                    out=accP[:], in0=vemb[:, base, :],
                    scalar1=probs[:, base:base + 1], scalar2=None, op0=A.mult,
                )
                for k in range(1, wp):
                    nc.gpsimd.tensor_scalar(
                        out=scrP[:], in0=vemb[:, base + k, :],
                        scalar1=probs[:, base + k:base + k + 1], scalar2=None,
                        op0=A.mult,
                    )
                    nc.gpsimd.tensor_tensor(
                        out=accP[:], in0=accP[:], in1=scrP[:], op=A.add,
                    )
                st["accP"] = accP

            def dve_prod_rowsums(t):
                st = S[t]
                kt = k_sched[t]
                rd, rc, wd, wc, wp = _splits(kt)
                kemb, q_t = st["kemb"], st["q"]
                logits = sp.tile([P, K], f32, tag="logits", name=f"logits_{t}")
                scrD = sp.tile([P, D], f16, tag="scrD", name=f"scrD_{t}")
                q_b = q_t[:].unsqueeze(1).broadcast_to([P, kt, D])
                nc.vector.tensor_tensor(
                    out=kemb[:, 0:kt, :], in0=kemb[:, 0:kt, :], in1=q_b, op=A.mult,
                )
                for k in range(rd):
                    nc.vector.tensor_scalar(
                        out=scrD[:], in0=kemb[:, k, :],
                        scalar1=1.0, scalar2=0.0, op0=A.mult, op1=A.add,
                        accum_out=logits[:, k:k + 1],
                    )
                st["logits"] = logits

            def act_rowsum_copies(t):
                st = S[t]
                kt = k_sched[t]
                rd, rc, wd, wc, wp = _splits(kt)
                kemb, logits = st["kemb"], st["logits"]
                scrA = sp.tile([P, D], f16, tag="scrA", name=f"scrA_{t}")
                for k in range(rd, kt):
                    nc.scalar.activation(
                        out=scrA[:], in_=kemb[:, k, :],
                        func=mybir.ActivationFunctionType.Copy,
                        bias=0.0, scale=1.0,
                        accum_out=logits[:, k:k + 1],
                    )

            def dve_bias_negmax(t):
                st = S[t]
                kt = k_sched[t]
                logits, bias_t = st["logits"], st["bias"]
                nc.vector.tensor_tensor(
                    out=logits[:, 0:kt], in0=logits[:, 0:kt], in1=bias_t[:, 0:kt],
                    op=A.add,
                )
                negmax = sp.tile([P, 1], f32, tag="negmax", name=f"negmax_{t}")
                nc.vector.tensor_reduce(
                    out=negmax[:], in_=logits[:, 0:kt], axis=mybir.AxisListType.X,
                    op=A.max, negate=True,
                )
                st["negmax"] = negmax

            def act_exp(t):
                st = S[t]
                kt = k_sched[t]
                probs = sp.tile([P, K], f32, tag="probs", name=f"probs_{t}")
                sumexp = sp.tile([P, 1], f32, tag="sumexp", name=f"sumexp_{t}")
                nc.scalar.activation(
                    out=probs[:, 0:kt], in_=st["logits"][:, 0:kt],
                    func=mybir.ActivationFunctionType.Exp,
                    bias=st["negmax"][:, :1], scale=1.0, accum_out=sumexp[:],
                )
                st["probs"] = probs
                st["sumexp"] = sumexp

            def dve_probscale(t):
                st = S[t]
                kt = k_sched[t]
                inv = sp.tile([P, 1], f32, tag="inv", name=f"inv_{t}")
                nc.vector.reciprocal(out=inv[:], in_=st["sumexp"][:])
                nc.vector.tensor_scalar(
                    out=st["probs"][:, 0:kt], in0=st["probs"][:, 0:kt],
                    scalar1=inv[:, :1], scalar2=None, op0=A.mult,
                )

            def dve_wsum(t):
                st = S[t]
                kt = k_sched[t]
                rd, rc, wd, wc, wp = _splits(kt)
                vemb, probs = st["vemb"], st["probs"]
                for k in range(wd):
                    nc.vector.tensor_scalar(
                        out=vemb[:, k, :], in0=vemb[:, k, :],
                        scalar1=probs[:, k:k + 1], scalar2=None, op0=A.mult,
                    )
                n = wd + wc
                while n > 1:
                    h = n // 2
                    nc.vector.tensor_tensor(
                        out=vemb[:, 0:h, :], in0=vemb[:, 0:h, :],
                        in1=vemb[:, n - h:n, :], op=A.add,
                    )
                    n = n - h

            def combine_store(t):
                st = S[t]
                out_t = sp.tile([P, D], f32, tag="out", name=f"out_{t}")
                if st["accP"] is None:
                    nc.vector.tensor_copy(out=out_t[:], in_=st["vemb"][:, 0, :])
                else:
                    nc.vector.tensor_tensor(
                        out=out_t[:], in0=st["vemb"][:, 0, :], in1=st["accP"][:],
                        op=A.add,
                    )
                rows = slice(t * P, (t + 1) * P)
                nc.sync.dma_start(out=out_d[rows, :], in_=out_t[:])

            loads(0)
            loads(1)
            gather_k(0)
            for t in range(NTILES):
                if t + 2 < NTILES:
                    loads(t + 2)
                if t + 1 < NTILES:
                    gather_k(t + 1)
                gather_v(t)
                if t - 1 >= 0:
                    act_wsum_copies(t - 1)
                    pool_wsum(t - 1)
                dve_prod_rowsums(t)
                act_rowsum_copies(t)
                dve_bias_negmax(t)
                if t - 1 >= 0:
                    dve_wsum(t - 1)
                act_exp(t)
                dve_probscale(t)
                if t - 1 >= 0:
                    combine_store(t - 1)
            t = NTILES - 1
            act_wsum_copies(t)
            pool_wsum(t)
            dve_wsum(t)
            combine_store(t)

    nc.compile()
    return nc


def _get_program(k_sched):
    k_sched = tuple(k_sched)
    if k_sched not in _PROGRAM_CACHE:
        _PROGRAM_CACHE[k_sched] = _build_program(k_sched)
    return _PROGRAM_CACHE[k_sched]


SLOT_ORDER = [0, 1, 2, 3]


def _plan_sort(pair_length):
    order = np.argsort(pair_length, kind="stable")
    blocks = order.reshape(NBLOCKS, P)
    perm = [[None] * NTILES for _ in range(N_CORES)]
    k_raw = [0] * NTILES
    for j in range(NTILES):
        base = SLOT_ORDER[j] * N_CORES
        for c in range(N_CORES):
            bi = base + (c if j % 2 == 0 else N_CORES - 1 - c)
            perm[c][j] = blocks[bi]
            k_raw[j] = max(k_raw[j], int(pair_length[blocks[bi]].max()))
    k_sched = [min(K, max(5, -(-k // 5) * 5)) for k in k_raw]
    return perm, k_sched


def _wrap_idx(idx_pk, kt):
    """idx_pk: [P, kt] int -> wrapped int16 [P, kt*8].

    dma_gather consumes index i = tile[i % 16, i // 16] with list position
    i = k*P + p landing at out[p, k, :]; the 16-partition block is
    replicated to all 128 partitions.
    """
    lst = idx_pk.T.reshape(-1)                       # i = k*P + p
    wrapped = lst.reshape(-1, 16).T.astype(np.int16)  # [16, kt*8]
    return np.tile(wrapped, (P // 16, 1))


def _prep_inputs(keys, values, pair_length, query, key_table, value_table):
    keys = np.asarray(keys).astype(np.int32)
    values = np.asarray(values).astype(np.int32)
    pair_length = np.asarray(pair_length).astype(np.int32)
    query = np.asarray(query, dtype=np.float32).astype(np.float16)
    key_table = np.asarray(key_table, dtype=np.float32).astype(np.float16)
    value_table = np.asarray(value_table, dtype=np.float32).astype(np.float16)

    bias = np.where(np.arange(K, dtype=np.int32)[None, :] < pair_length[:, None],
                    np.float32(0.0), MASK_NEG).astype(np.float32)

    perm, k_sched = _plan_sort(pair_length)
    in_maps = []
    core_ids_order = []
    for c in range(N_CORES):
        ids = np.concatenate(perm[c])
        core_ids_order.append(ids)
        keys_c = keys[ids]
        values_c = values[ids]

        used_k = [keys_c[t * P:(t + 1) * P, 0:k_sched[t]] for t in range(NTILES)]
        used_v = [values_c[t * P:(t + 1) * P, 0:k_sched[t]] for t in range(NTILES)]
        uniq_k, inv_k = np.unique(np.concatenate([u.ravel() for u in used_k]),
                                  return_inverse=True)
        uniq_v, inv_v = np.unique(np.concatenate([u.ravel() for u in used_v]),
                                  return_inverse=True)
        ktab_c = np.zeros((CAP, D), dtype=np.float16)
        ktab_c[:len(uniq_k)] = key_table[uniq_k]
        vtab_c = np.zeros((CAP, D), dtype=np.float16)
        vtab_c[:len(uniq_v)] = value_table[uniq_v]

        kidx_parts, vidx_parts = [], []
        off_k = off_v = 0
        for t in range(NTILES):
            kt = k_sched[t]
            n = P * kt
            kidx_parts.append(_wrap_idx(inv_k[off_k:off_k + n].reshape(P, kt), kt))
            vidx_parts.append(_wrap_idx(inv_v[off_v:off_v + n].reshape(P, kt), kt))
            off_k += n
            off_v += n
        in_maps.append({
            "kidx": np.ascontiguousarray(np.concatenate(kidx_parts, axis=1)),
            "vidx": np.ascontiguousarray(np.concatenate(vidx_parts, axis=1)),
            "bias": np.ascontiguousarray(bias[ids]),
            "query": np.ascontiguousarray(query[ids]),
            "key_table": ktab_c,
            "value_table": vtab_c,
        })
    return in_maps, core_ids_order, k_sched


def kernel(keys, values, pair_length, query, key_table, value_table):
    in_maps, core_ids_order, k_sched = _prep_inputs(
        keys, values, pair_length, query, key_table, value_table)
    nc = _get_program(k_sched)
    res = bass_utils.run_bass_kernel_spmd(nc, in_maps, core_ids=list(range(N_CORES)))
    out = np.empty((B, D), dtype=np.float32)
    for c in range(N_CORES):
        out[core_ids_order[c]] = res.results[c]["out"]
    return out


# revision 8
# speedup vs baseline: 1.1485x; 1.1485x over previous
"""KVMemory kernel for Trainium2 (8 NeuronCores, Bass/Tile).

Strategy (v14):
  - Data-parallel over batch. The host sorts the 4096 examples by
    pair_length, deals 128-example blocks snake-wise to the 8 cores, and
    tile slot j computes only k_sched[j] <= 50 keys (pair_length averages
    ~25 of 50, so half the key/value positions are softmax-masked; the
    sort turns that waste into real savings). Outputs are inverse-permuted
    on the host.
  - Embedding rows are fetched with the ANT dma_gather instruction in
    1024-row chunks (the SWDGE descriptor ring holds 1024 descriptors; a
    single larger gather wedges the Q7 ucode -- found the hard way).
    That is still ~50x fewer SWDGE instructions than per-column indirect
    DMA, whose multi-column offset form returns wrong data on hardware.
    dma_gather's int16 index limit is satisfied by host-side per-core
    table compaction: each core touches <= 25600 < 2^15 unique rows per
    table, so the host dedups (np.unique) and ships a compact fp16 table
    plus pre-wrapped int16 index tensors.
  - Three-engine compute split per tile (all ops verified against the
    neuronxcc ISA check and hardware):
      DVE:  broadcast-TT q*kemb product (2x perf mode), tensor_scalar 4x
            rowsums and scales, 2x tree adds, softmax small ops
      ACT:  activation(Copy) rowsums + wsum scales, Exp
      Pool: gather emission, wsum tensor_scalar+tensor_tensor ping-pong
  - Software pipelining: gathers for tile t+1 and the weighted sum of
    tile t-1 interleave with tile t's logits/softmax, ordered so no
    in-order engine queue blocks on unready work.
"""

import sys

if "/opt/trn_rl_repo" not in sys.path:
    sys.path.insert(0, "/opt/trn_rl_repo")

import numpy as np

import concourse.bass as bass
import concourse.mybir as mybir
import concourse.tile as tile
from concourse import bacc
from concourse import library_config
import concourse.bass_utils as bass_utils

N_CORES = 8
B = 4096
K = 50
D = 256
NUM_KEYS = 100000
NUM_VALUES = 100000
BC = B // N_CORES
P = 128
NTILES = BC // P
NBLOCKS = B // P
CAP = BC * K            # compact table rows per core (worst case, < 2^15)
MASK_NEG = np.float32(-1e30)

R_ACT = 0.40    # fraction of rowsums on ACT (rest on DVE)
W_ACT = 0.35    # fraction of wsum scales on ACT (tree on DVE)
W_POOL = 0.0    # base Pool wsum fraction (per-slot override below)
W_POOL_SLOT = [0.10, 0.20, 0.15, 0.0]  # Pool emits finish early; late tiles borrow it

_PROGRAM_CACHE = {}


def _splits(kt, slot=None):
    """Returns (rd, rc, wd, wc, wp): rowsum DVE/ACT counts; wsum DVE/ACT/Pool."""
    rc = int(round(kt * R_ACT))
    rd = kt - rc
    wfrac = W_POOL if slot is None else W_POOL_SLOT[slot]
    wp = int(round(kt * wfrac))
    wc = int(round(kt * W_ACT))
    wd = kt - wp - wc
    if wd < 1:
        wd = 1
        wc = max(0, kt - wd - wp)
        wp = kt - wd - wc
    return rd, rc, wd, wc, wp

GCHUNK = 8  # slabs per dma_gather (1024 rows; SWDGE ring holds 1024 descs)


def _build_program(k_sched):
    f16 = mybir.dt.float16
    f32 = mybir.dt.float32
    i16 = mybir.dt.int16
    A = mybir.AluOpType
    KMAX = max(k_sched)
    ksum = sum(k_sched)
    nc = bacc.Bacc("TRN2", target_bir_lowering=False, debug=False, num_devices=N_CORES)

    kidx_d = nc.dram_tensor("kidx", [P, ksum * 8], i16, kind="ExternalInput")
    vidx_d = nc.dram_tensor("vidx", [P, ksum * 8], i16, kind="ExternalInput")
    bias_d = nc.dram_tensor("bias", [BC, K], f32, kind="ExternalInput")
    query_d = nc.dram_tensor("query", [BC, D], f16, kind="ExternalInput")
    ktab_d = nc.dram_tensor("key_table", [CAP, D], f16, kind="ExternalInput")
    vtab_d = nc.dram_tensor("value_table", [CAP, D], f16, kind="ExternalInput")
    out_d = nc.dram_tensor("out", [BC, D], f32, kind="ExternalOutput")

    idx_off = [0]
    for kt in k_sched:
        idx_off.append(idx_off[-1] + kt * 8)

    with tile.TileContext(nc) as tc:
        with (
            tc.tile_pool(name="kpool", bufs=2) as kpool,
            tc.tile_pool(name="vpool", bufs=3) as vpool,
            tc.tile_pool(name="small", bufs=3) as sp,
        ):
            S = {}
            nc.gpsimd.load_library(library_config.mlp)

            def loads(t):
                kt = k_sched[t]
                rows = slice(t * P, (t + 1) * P)
                cols = slice(idx_off[t], idx_off[t + 1])
                kidx_t = sp.tile([P, KMAX * 8], i16, tag="kidx", name=f"kidx_{t}")
                nc.sync.dma_start(out=kidx_t[:, 0:kt * 8], in_=kidx_d[:, cols])
                vidx_t = sp.tile([P, KMAX * 8], i16, tag="vidx", name=f"vidx_{t}")
                nc.sync.dma_start(out=vidx_t[:, 0:kt * 8], in_=vidx_d[:, cols])
                bias_t = sp.tile([P, K], f32, tag="bias", name=f"bias_{t}")
                nc.sync.dma_start(out=bias_t[:, 0:kt], in_=bias_d[rows, 0:kt])
                q_t = sp.tile([P, D], f16, tag="q", name=f"q_{t}")
                nc.sync.dma_start(out=q_t[:], in_=query_d[rows, :])
                S[t] = dict(kidx=kidx_t, vidx=vidx_t, bias=bias_t, q=q_t)

            def _gather_chunks(dst, tab, idx, kt):
                for lo in range(0, kt, GCHUNK):
                    hi = min(lo + GCHUNK, kt)
                    n = P * (hi - lo)
                    nc.gpsimd.dma_gather(
                        dst[:, lo:hi, :], tab[:], idx[:, lo * 8:hi * 8], n, n, D,
                    )

            def gather_k(t):
                st = S[t]
                kt = k_sched[t]
                kemb = kpool.tile([P, KMAX, D], f16, tag="kemb", name=f"kemb_{t}")
                _gather_chunks(kemb, ktab_d, st["kidx"], kt)
                st["kemb"] = kemb

            def gather_v(t):
                st = S[t]
                kt = k_sched[t]
                vemb = vpool.tile([P, KMAX, D], f16, tag="vemb", name=f"vemb_{t}")
                _gather_chunks(vemb, vtab_d, st["vidx"], kt)
                st["vemb"] = vemb

            def act_wsum_copies(t):
                st = S[t]
                rd, rc, wd, wc, wp = _splits(k_sched[t], t)
                vemb, probs = st["vemb"], st["probs"]
                for k in range(wd, wd + wc):
                    nc.scalar.activation(
                        out=vemb[:, k, :], in_=vemb[:, k, :],
                        func=mybir.ActivationFunctionType.Copy,
                        bias=0.0, scale=probs[:, k:k + 1],
                    )

            def pool_wsum(t):
                st = S[t]
                kt = k_sched[t]
                rd, rc, wd, wc, wp = _splits(kt, t)
                if wp == 0:
                    st["accP"] = None
                    return
                vemb, probs = st["vemb"], st["probs"]
                base = wd + wc
                accP = sp.tile([P, D], f32, tag="accP", name=f"accP_{t}")
                scrP = sp.tile([P, D], f32, tag="scrP", name=f"scrP_{t}")
                nc.gpsimd.tensor_scalar(
                    out=accP[:], in0=vemb[:, base, :],
                    scalar1=probs[:, base:base + 1], scalar2=None, op0=A.mult,
                )
                for k in range(1, wp):
                    nc.gpsimd.tensor_scalar(
                        out=scrP[:], in0=vemb[:, base + k, :],
                        scalar1=probs[:, base + k:base + k + 1], scalar2=None,
                        op0=A.mult,
                    )
                    nc.gpsimd.tensor_tensor(
                        out=accP[:], in0=accP[:], in1=scrP[:], op=A.add,
                    )
                st["accP"] = accP

            def dve_prod_rowsums(t):
                st = S[t]
                kt = k_sched[t]
                rd, rc, wd, wc, wp = _splits(kt)
                kemb, q_t = st["kemb"], st["q"]
                logits = sp.tile([P, K], f32, tag="logits", name=f"logits_{t}")
                scrD = sp.tile([P, D], f16, tag="scrD", name=f"scrD_{t}")
                for lo in range(0, kt, GCHUNK):
                    hi = min(lo + GCHUNK, kt)
                    q_b = q_t[:].unsqueeze(1).broadcast_to([P, hi - lo, D])
                    nc.vector.tensor_tensor(
                        out=kemb[:, lo:hi, :], in0=kemb[:, lo:hi, :], in1=q_b,
                        op=A.mult,
                    )
                    for k in range(lo, min(hi, rd)):
                        nc.vector.tensor_scalar(
                            out=scrD[:], in0=kemb[:, k, :],
                            scalar1=1.0, scalar2=0.0, op0=A.mult, op1=A.add,
                            accum_out=logits[:, k:k + 1],
                        )
                st["logits"] = logits

            def act_rowsum_copies(t):
                st = S[t]
                kt = k_sched[t]
                rd, rc, wd, wc, wp = _splits(kt)
                kemb, logits = st["kemb"], st["logits"]
                scrA = sp.tile([P, D], f16, tag="scrA", name=f"scrA_{t}")
                for k in range(rd, kt):
                    nc.scalar.activation(
                        out=scrA[:], in_=kemb[:, k, :],
                        func=mybir.ActivationFunctionType.Copy,
                        bias=0.0, scale=1.0,
                        accum_out=logits[:, k:k + 1],
                    )

            def dve_bias_negmax(t):
                st = S[t]
                kt = k_sched[t]
                logits, bias_t = st["logits"], st["bias"]
                nc.vector.tensor_tensor(
                    out=logits[:, 0:kt], in0=logits[:, 0:kt], in1=bias_t[:, 0:kt],
                    op=A.add,
                )
                negmax = sp.tile([P, 1], f32, tag="negmax", name=f"negmax_{t}")
                nc.vector.tensor_reduce(
                    out=negmax[:], in_=logits[:, 0:kt], axis=mybir.AxisListType.X,
                    op=A.max, negate=True,
                )
                st["negmax"] = negmax

            def act_exp(t):
                st = S[t]
                kt = k_sched[t]
                probs = sp.tile([P, K], f32, tag="probs", name=f"probs_{t}")
                sumexp = sp.tile([P, 1], f32, tag="sumexp", name=f"sumexp_{t}")
                nc.scalar.activation(
                    out=probs[:, 0:kt], in_=st["logits"][:, 0:kt],
                    func=mybir.ActivationFunctionType.Exp,
                    bias=st["negmax"][:, :1], scale=1.0, accum_out=sumexp[:],
                )
                st["probs"] = probs
                st["sumexp"] = sumexp

            def dve_probscale(t):
                st = S[t]
                kt = k_sched[t]
                inv = sp.tile([P, 1], f32, tag="inv", name=f"inv_{t}")
                nc.vector.reciprocal(out=inv[:], in_=st["sumexp"][:])
                nc.vector.tensor_scalar(
                    out=st["probs"][:, 0:kt], in0=st["probs"][:, 0:kt],
                    scalar1=inv[:, :1], scalar2=None, op0=A.mult,
                )

            def dve_wsum(t):
                st = S[t]
                kt = k_sched[t]
                rd, rc, wd, wc, wp = _splits(kt, t)
                vemb, probs = st["vemb"], st["probs"]
                for k in range(wd):
                    nc.vector.tensor_scalar(
                        out=vemb[:, k, :], in0=vemb[:, k, :],
                        scalar1=probs[:, k:k + 1], scalar2=None, op0=A.mult,
                    )
                n = wd + wc
                while n > 1:
                    h = n // 2
                    nc.vector.tensor_tensor(
                        out=vemb[:, 0:h, :], in0=vemb[:, 0:h, :],
                        in1=vemb[:, n - h:n, :], op=A.add,
                    )
                    n = n - h

            def combine_store(t):
                st = S[t]
                out_t = sp.tile([P, D], f32, tag="out", name=f"out_{t}")
                if st["accP"] is None:
                    nc.vector.tensor_copy(out=out_t[:], in_=st["vemb"][:, 0, :])
                else:
                    nc.vector.tensor_tensor(
                        out=out_t[:], in0=st["vemb"][:, 0, :], in1=st["accP"][:],
                        op=A.add,
                    )
                rows = slice(t * P, (t + 1) * P)
                nc.sync.dma_start(out=out_d[rows, :], in_=out_t[:])

            loads(0)
            loads(1)
            gather_k(0)
            for t in range(NTILES):
                if t + 2 < NTILES:
                    loads(t + 2)
                if t + 1 < NTILES:
                    gather_k(t + 1)
                gather_v(t)
                if t - 1 >= 0:
                    act_wsum_copies(t - 1)
                    pool_wsum(t - 1)
                dve_prod_rowsums(t)
                act_rowsum_copies(t)
                dve_bias_negmax(t)
                if t - 1 >= 0:
                    dve_wsum(t - 1)
                act_exp(t)
                dve_probscale(t)
                if t - 1 >= 0:
                    combine_store(t - 1)
            t = NTILES - 1
            act_wsum_copies(t)
            pool_wsum(t)
            dve_wsum(t)
            combine_store(t)

    nc.compile()
    return nc


def _get_program(k_sched):
    k_sched = tuple(k_sched)
    if k_sched not in _PROGRAM_CACHE:
        _PROGRAM_CACHE[k_sched] = _build_program(k_sched)
    return _PROGRAM_CACHE[k_sched]


SLOT_ORDER = [0, 2, 3, 1]


def _plan_sort(pair_length):
    order = np.argsort(pair_length, kind="stable")
    blocks = order.reshape(NBLOCKS, P)
    perm = [[None] * NTILES for _ in range(N_CORES)]
    k_raw = [0] * NTILES
    for j in range(NTILES):
        base = SLOT_ORDER[j] * N_CORES
        for c in range(N_CORES):
            bi = base + (c if j % 2 == 0 else N_CORES - 1 - c)
            perm[c][j] = blocks[bi]
            k_raw[j] = max(k_raw[j], int(pair_length[blocks[bi]].max()))
    k_sched = [min(K, max(4, k)) for k in k_raw]
    return perm, k_sched


def _wrap_idx(idx_pk, kt):
    """idx_pk: [P, kt] int -> wrapped int16 [P, kt*8].

    dma_gather consumes index i = tile[i % 16, i // 16] with list position
    i = k*P + p landing at out[p, k, :]; the 16-partition block is
    replicated to all 128 partitions.
    """
    lst = idx_pk.T.reshape(-1)                       # i = k*P + p
    wrapped = lst.reshape(-1, 16).T.astype(np.int16)  # [16, kt*8]
    return np.tile(wrapped, (P // 16, 1))


def _prep_inputs(keys, values, pair_length, query, key_table, value_table):
    keys = np.asarray(keys).astype(np.int32)
    values = np.asarray(values).astype(np.int32)
    pair_length = np.asarray(pair_length).astype(np.int32)
    query = np.asarray(query, dtype=np.float32).astype(np.float16)
    key_table = np.asarray(key_table, dtype=np.float32).astype(np.float16)
    value_table = np.asarray(value_table, dtype=np.float32).astype(np.float16)

    bias = np.where(np.arange(K, dtype=np.int32)[None, :] < pair_length[:, None],
                    np.float32(0.0), MASK_NEG).astype(np.float32)

    perm, k_sched = _plan_sort(pair_length)
    in_maps = []
    core_ids_order = []
    for c in range(N_CORES):
        ids = np.concatenate(perm[c])
        core_ids_order.append(ids)
        keys_c = keys[ids]
        values_c = values[ids]

        used_k = [keys_c[t * P:(t + 1) * P, 0:k_sched[t]] for t in range(NTILES)]
        used_v = [values_c[t * P:(t + 1) * P, 0:k_sched[t]] for t in range(NTILES)]
        uniq_k, inv_k = np.unique(np.concatenate([u.ravel() for u in used_k]),
                                  return_inverse=True)
        uniq_v, inv_v = np.unique(np.concatenate([u.ravel() for u in used_v]),
                                  return_inverse=True)
        ktab_c = np.zeros((CAP, D), dtype=np.float16)
        ktab_c[:len(uniq_k)] = key_table[uniq_k]
        vtab_c = np.zeros((CAP, D), dtype=np.float16)
        vtab_c[:len(uniq_v)] = value_table[uniq_v]

        kidx_parts, vidx_parts = [], []
        off_k = off_v = 0
        for t in range(NTILES):
            kt = k_sched[t]
            n = P * kt
            kidx_parts.append(_wrap_idx(inv_k[off_k:off_k + n].reshape(P, kt), kt))
            vidx_parts.append(_wrap_idx(inv_v[off_v:off_v + n].reshape(P, kt), kt))
            off_k += n
            off_v += n
        in_maps.append({
            "kidx": np.ascontiguousarray(np.concatenate(kidx_parts, axis=1)),
            "vidx": np.ascontiguousarray(np.concatenate(vidx_parts, axis=1)),
            "bias": np.ascontiguousarray(bias[ids]),
            "query": np.ascontiguousarray(query[ids]),
            "key_table": ktab_c,
            "value_table": vtab_c,
        })
    return in_maps, core_ids_order, k_sched


def kernel(keys, values, pair_length, query, key_table, value_table):
    in_maps, core_ids_order, k_sched = _prep_inputs(
        keys, values, pair_length, query, key_table, value_table)
    nc = _get_program(k_sched)
    res = bass_utils.run_bass_kernel_spmd(nc, in_maps, core_ids=list(range(N_CORES)))
    out = np.empty((B, D), dtype=np.float32)
    for c in range(N_CORES):
        out[core_ids_order[c]] = res.results[c]["out"]
    return out


# revision 9
# speedup vs baseline: 1.1616x; 1.0115x over previous
"""KVMemory kernel for Trainium2 (8 NeuronCores, Bass/Tile).

Strategy (v17):
  - Data-parallel over batch. The host sorts the 4096 examples by
    pair_length, deals 128-example blocks snake-wise to the 8 cores, and
    tile slot j computes only k_sched[j] <= 50 keys (pair_length averages
    ~25 of 50, so half the key/value positions are softmax-masked; the
    sort turns that waste into real savings). Outputs are inverse-permuted
    on the host.
  - Embedding rows are fetched with the ANT dma_gather instruction in
    1024-row chunks (the SWDGE descriptor ring holds 1024 descriptors; a
    single larger gather wedges the Q7 ucode -- found the hard way).
    That is still ~50x fewer SWDGE instructions than per-column indirect
    DMA, whose multi-column offset form returns wrong data on hardware.
    dma_gather's int16 index limit is satisfied by host-side per-core
    table compaction: each core touches <= 25600 < 2^15 unique rows per
    table, so the host dedups (np.unique) and ships a compact fp16 table
    plus pre-wrapped int16 index tensors.
  - Three-engine compute split per tile (all ops verified against the
    neuronxcc ISA check and hardware):
      DVE:  broadcast-TT q*kemb product (2x perf mode), tensor_scalar 4x
            rowsums and scales, 2x tree adds, softmax small ops
      ACT:  activation(Copy) rowsums + wsum scales, Exp
      Pool: gather emission, wsum tensor_scalar+tensor_tensor ping-pong
  - Software pipelining: gathers for tile t+1 and the weighted sum of
    tile t-1 interleave with tile t's logits/softmax, ordered so no
    in-order engine queue blocks on unready work.
"""

import sys

if "/opt/trn_rl_repo" not in sys.path:
    sys.path.insert(0, "/opt/trn_rl_repo")

import numpy as np

import concourse.bass as bass
import concourse.mybir as mybir
import concourse.tile as tile
from concourse import bacc
from concourse import library_config
import concourse.bass_utils as bass_utils

N_CORES = 8
B = 4096
K = 50
D = 256
NUM_KEYS = 100000
NUM_VALUES = 100000
BC = B // N_CORES
P = 128
NTILES = BC // P
NBLOCKS = B // P
CAP = BC * K            # compact table rows per core (worst case, < 2^15)
MASK_NEG = np.float32(-1e30)

R_ACT = 0.40    # fraction of rowsums on ACT (rest on DVE)
W_ACT = 0.35    # fraction of wsum scales on ACT (tree on DVE)
W_POOL = 0.0    # base Pool wsum fraction (per-slot override below)
W_POOL_SLOT = [0.10, 0.20, 0.15, 0.0]  # Pool emits finish early; late tiles borrow it

_PROGRAM_CACHE = {}


def _splits(kt, slot=None):
    """Returns (rd, rc, wd, wc, wp): rowsum DVE/ACT counts; wsum DVE/ACT/Pool."""
    rc = int(round(kt * R_ACT))
    rd = kt - rc
    wfrac = W_POOL if slot is None else W_POOL_SLOT[slot]
    wp = int(round(kt * wfrac))
    wc = int(round(kt * W_ACT))
    wd = kt - wp - wc
    if wd < 1:
        wd = 1
        wc = max(0, kt - wd - wp)
        wp = kt - wd - wc
    return rd, rc, wd, wc, wp

GCHUNK = 8  # slabs per dma_gather (1024 rows; SWDGE ring holds 1024 descs)


def _build_program(k_sched):
    f16 = mybir.dt.float16
    f32 = mybir.dt.float32
    i16 = mybir.dt.int16
    A = mybir.AluOpType
    KMAX = max(k_sched)
    ksum = sum(k_sched)
    nc = bacc.Bacc("TRN2", target_bir_lowering=False, debug=False, num_devices=N_CORES)

    kidx_d = nc.dram_tensor("kidx", [P, ksum * 8], i16, kind="ExternalInput")
    vidx_d = nc.dram_tensor("vidx", [P, ksum * 8], i16, kind="ExternalInput")
    bias_d = nc.dram_tensor("bias", [BC, K], f32, kind="ExternalInput")
    query_d = nc.dram_tensor("query", [BC, D], f16, kind="ExternalInput")
    ktab_d = nc.dram_tensor("key_table", [CAP, D], f16, kind="ExternalInput")
    vtab_d = nc.dram_tensor("value_table", [CAP, D], f16, kind="ExternalInput")
    out_d = nc.dram_tensor("out", [BC, D], f32, kind="ExternalOutput")

    idx_off = [0]
    for kt in k_sched:
        idx_off.append(idx_off[-1] + kt * 8)

    with tile.TileContext(nc) as tc:
        with (
            tc.tile_pool(name="kpool", bufs=2) as kpool,
            tc.tile_pool(name="vpool", bufs=3) as vpool,
            tc.tile_pool(name="small", bufs=3) as sp,
        ):
            S = {}
            nc.gpsimd.load_library(library_config.mlp)

            def loads(t):
                kt = k_sched[t]
                rows = slice(t * P, (t + 1) * P)
                cols = slice(idx_off[t], idx_off[t + 1])
                kidx_t = sp.tile([P, KMAX * 8], i16, tag="kidx", name=f"kidx_{t}")
                nc.sync.dma_start(out=kidx_t[:, 0:kt * 8], in_=kidx_d[:, cols])
                vidx_t = sp.tile([P, KMAX * 8], i16, tag="vidx", name=f"vidx_{t}")
                nc.sync.dma_start(out=vidx_t[:, 0:kt * 8], in_=vidx_d[:, cols])
                bias_t = sp.tile([P, K], f32, tag="bias", name=f"bias_{t}")
                nc.sync.dma_start(out=bias_t[:, 0:kt], in_=bias_d[rows, 0:kt])
                q_t = sp.tile([P, D], f16, tag="q", name=f"q_{t}")
                nc.sync.dma_start(out=q_t[:], in_=query_d[rows, :])
                S[t] = dict(kidx=kidx_t, vidx=vidx_t, bias=bias_t, q=q_t)

            def _gather_chunks(dst, tab, idx, kt):
                for lo in range(0, kt, GCHUNK):
                    hi = min(lo + GCHUNK, kt)
                    n = P * (hi - lo)
                    nc.gpsimd.dma_gather(
                        dst[:, lo:hi, :], tab[:], idx[:, lo * 8:hi * 8], n, n, D,
                    )

            def gather_k(t):
                st = S[t]
                kt = k_sched[t]
                kemb = kpool.tile([P, KMAX, D], f16, tag="kemb", name=f"kemb_{t}")
                _gather_chunks(kemb, ktab_d, st["kidx"], kt)
                st["kemb"] = kemb

            def gather_v(t):
                st = S[t]
                kt = k_sched[t]
                vemb = vpool.tile([P, KMAX, D], f16, tag="vemb", name=f"vemb_{t}")
                _gather_chunks(vemb, vtab_d, st["vidx"], kt)
                st["vemb"] = vemb

            def act_wsum_copies(t):
                st = S[t]
                rd, rc, wd, wc, wp = _splits(k_sched[t], t)
                vemb, probs = st["vemb"], st["probs"]
                for k in range(wd, wd + wc):
                    nc.scalar.activation(
                        out=vemb[:, k, :], in_=vemb[:, k, :],
                        func=mybir.ActivationFunctionType.Copy,
                        bias=0.0, scale=probs[:, k:k + 1],
                    )

            def pool_wsum(t):
                st = S[t]
                kt = k_sched[t]
                rd, rc, wd, wc, wp = _splits(kt, t)
                if wp == 0:
                    st["accP"] = None
                    return
                vemb, probs = st["vemb"], st["probs"]
                base = wd + wc
                accP = sp.tile([P, D], f32, tag="accP", name=f"accP_{t}")
                scrP = sp.tile([P, D], f32, tag="scrP", name=f"scrP_{t}")
                nc.gpsimd.tensor_scalar(
                    out=accP[:], in0=vemb[:, base, :],
                    scalar1=probs[:, base:base + 1], scalar2=None, op0=A.mult,
                )
                for k in range(1, wp):
                    nc.gpsimd.tensor_scalar(
                        out=scrP[:], in0=vemb[:, base + k, :],
                        scalar1=probs[:, base + k:base + k + 1], scalar2=None,
                        op0=A.mult,
                    )
                    nc.gpsimd.tensor_tensor(
                        out=accP[:], in0=accP[:], in1=scrP[:], op=A.add,
                    )
                st["accP"] = accP

            def dve_prod_rowsums(t):
                st = S[t]
                kt = k_sched[t]
                rd, rc, wd, wc, wp = _splits(kt)
                kemb, q_t = st["kemb"], st["q"]
                logits = sp.tile([P, K], f32, tag="logits", name=f"logits_{t}")
                scrD = sp.tile([P, D], f16, tag="scrD", name=f"scrD_{t}")
                for lo in range(0, kt, GCHUNK):
                    hi = min(lo + GCHUNK, kt)
                    q_b = q_t[:].unsqueeze(1).broadcast_to([P, hi - lo, D])
                    nc.vector.tensor_tensor(
                        out=kemb[:, lo:hi, :], in0=kemb[:, lo:hi, :], in1=q_b,
                        op=A.mult,
                    )
                    for k in range(lo, min(hi, rd)):
                        nc.vector.tensor_scalar(
                            out=scrD[:], in0=kemb[:, k, :],
                            scalar1=1.0, scalar2=0.0, op0=A.mult, op1=A.add,
                            accum_out=logits[:, k:k + 1],
                        )
                st["logits"] = logits

            def act_rowsum_copies(t):
                st = S[t]
                kt = k_sched[t]
                rd, rc, wd, wc, wp = _splits(kt)
                kemb, logits = st["kemb"], st["logits"]
                scrA = sp.tile([P, D], f16, tag="scrA", name=f"scrA_{t}")
                for k in range(rd, kt):
                    nc.scalar.activation(
                        out=scrA[:], in_=kemb[:, k, :],
                        func=mybir.ActivationFunctionType.Copy,
                        bias=0.0, scale=1.0,
                        accum_out=logits[:, k:k + 1],
                    )

            def dve_bias_negmax(t):
                st = S[t]
                kt = k_sched[t]
                logits, bias_t = st["logits"], st["bias"]
                nc.vector.tensor_tensor(
                    out=logits[:, 0:kt], in0=logits[:, 0:kt], in1=bias_t[:, 0:kt],
                    op=A.add,
                )
                negmax = sp.tile([P, 1], f32, tag="negmax", name=f"negmax_{t}")
                nc.vector.tensor_reduce(
                    out=negmax[:], in_=logits[:, 0:kt], axis=mybir.AxisListType.X,
                    op=A.max, negate=True,
                )
                st["negmax"] = negmax

            def act_exp(t):
                st = S[t]
                kt = k_sched[t]
                probs = sp.tile([P, K], f32, tag="probs", name=f"probs_{t}")
                sumexp = sp.tile([P, 1], f32, tag="sumexp", name=f"sumexp_{t}")
                nc.scalar.activation(
                    out=probs[:, 0:kt], in_=st["logits"][:, 0:kt],
                    func=mybir.ActivationFunctionType.Exp,
                    bias=st["negmax"][:, :1], scale=1.0, accum_out=sumexp[:],
                )
                st["probs"] = probs
                st["sumexp"] = sumexp

            def dve_recip(t):
                st = S[t]
                inv = sp.tile([P, 1], f32, tag="inv", name=f"inv_{t}")
                nc.vector.reciprocal(out=inv[:], in_=st["sumexp"][:])
                st["inv"] = inv

            def dve_wsum(t):
                st = S[t]
                kt = k_sched[t]
                rd, rc, wd, wc, wp = _splits(kt, t)
                vemb, probs = st["vemb"], st["probs"]
                for k in range(wd):
                    nc.vector.tensor_scalar(
                        out=vemb[:, k, :], in0=vemb[:, k, :],
                        scalar1=probs[:, k:k + 1], scalar2=None, op0=A.mult,
                    )
                n = wd + wc
                while n > 1:
                    h = n // 2
                    nc.vector.tensor_tensor(
                        out=vemb[:, 0:h, :], in0=vemb[:, 0:h, :],
                        in1=vemb[:, n - h:n, :], op=A.add,
                    )
                    n = n - h

            def combine_store(t):
                st = S[t]
                out_t = sp.tile([P, D], f32, tag="out", name=f"out_{t}")
                inv = st["inv"]
                if st["accP"] is None:
                    nc.vector.tensor_scalar(
                        out=out_t[:], in0=st["vemb"][:, 0, :],
                        scalar1=inv[:, :1], scalar2=None, op0=A.mult,
                    )
                else:
                    nc.vector.tensor_tensor(
                        out=out_t[:], in0=st["vemb"][:, 0, :], in1=st["accP"][:],
                        op=A.add,
                    )
                    nc.vector.tensor_scalar(
                        out=out_t[:], in0=out_t[:],
                        scalar1=inv[:, :1], scalar2=None, op0=A.mult,
                    )
                rows = slice(t * P, (t + 1) * P)
                nc.sync.dma_start(out=out_d[rows, :], in_=out_t[:])

            loads(0)
            loads(1)
            gather_k(0)
            for t in range(NTILES):
                if t + 2 < NTILES:
                    loads(t + 2)
                if t + 1 < NTILES:
                    gather_k(t + 1)
                gather_v(t)
                if t - 1 >= 0:
                    act_wsum_copies(t - 1)
                    pool_wsum(t - 1)
                dve_prod_rowsums(t)
                act_rowsum_copies(t)
                dve_bias_negmax(t)
                if t - 1 >= 0:
                    dve_wsum(t - 1)
                act_exp(t)
                dve_recip(t)
                if t - 1 >= 0:
                    combine_store(t - 1)
            t = NTILES - 1
            act_wsum_copies(t)
            pool_wsum(t)
            dve_wsum(t)
            combine_store(t)

    nc.compile()
    return nc


def _get_program(k_sched):
    k_sched = tuple(k_sched)
    if k_sched not in _PROGRAM_CACHE:
        _PROGRAM_CACHE[k_sched] = _build_program(k_sched)
    return _PROGRAM_CACHE[k_sched]


SLOT_ORDER = [0, 2, 3, 1]


def _plan_sort(pair_length):
    order = np.argsort(pair_length, kind="stable")
    blocks = order.reshape(NBLOCKS, P)
    perm = [[None] * NTILES for _ in range(N_CORES)]
    k_raw = [0] * NTILES
    for j in range(NTILES):
        base = SLOT_ORDER[j] * N_CORES
        for c in range(N_CORES):
            bi = base + (c if j % 2 == 0 else N_CORES - 1 - c)
            perm[c][j] = blocks[bi]
            k_raw[j] = max(k_raw[j], int(pair_length[blocks[bi]].max()))
    k_sched = [min(K, max(4, k)) for k in k_raw]
    return perm, k_sched


def _wrap_idx(idx_pk, kt):
    """idx_pk: [P, kt] int -> wrapped int16 [P, kt*8].

    dma_gather consumes index i = tile[i % 16, i // 16] with list position
    i = k*P + p landing at out[p, k, :]; the 16-partition block is
    replicated to all 128 partitions.
    """
    lst = idx_pk.T.reshape(-1)                       # i = k*P + p
    wrapped = lst.reshape(-1, 16).T.astype(np.int16)  # [16, kt*8]
    return np.tile(wrapped, (P // 16, 1))


def _prep_inputs(keys, values, pair_length, query, key_table, value_table):
    keys = np.asarray(keys).astype(np.int32)
    values = np.asarray(values).astype(np.int32)
    pair_length = np.asarray(pair_length).astype(np.int32)
    query = np.asarray(query, dtype=np.float32).astype(np.float16)
    key_table = np.asarray(key_table, dtype=np.float32).astype(np.float16)
    value_table = np.asarray(value_table, dtype=np.float32).astype(np.float16)

    bias = np.where(np.arange(K, dtype=np.int32)[None, :] < pair_length[:, None],
                    np.float32(0.0), MASK_NEG).astype(np.float32)

    perm, k_sched = _plan_sort(pair_length)
    in_maps = []
    core_ids_order = []
    for c in range(N_CORES):
        ids = np.concatenate(perm[c])
        core_ids_order.append(ids)
        keys_c = keys[ids]
        values_c = values[ids]

        used_k = [keys_c[t * P:(t + 1) * P, 0:k_sched[t]] for t in range(NTILES)]
        used_v = [values_c[t * P:(t + 1) * P, 0:k_sched[t]] for t in range(NTILES)]
        uniq_k, inv_k = np.unique(np.concatenate([u.ravel() for u in used_k]),
                                  return_inverse=True)
        uniq_v, inv_v = np.unique(np.concatenate([u.ravel() for u in used_v]),
                                  return_inverse=True)
        ktab_c = np.zeros((CAP, D), dtype=np.float16)
        ktab_c[:len(uniq_k)] = key_table[uniq_k]
        vtab_c = np.zeros((CAP, D), dtype=np.float16)
        vtab_c[:len(uniq_v)] = value_table[uniq_v]

        kidx_parts, vidx_parts = [], []
        off_k = off_v = 0
        for t in range(NTILES):
            kt = k_sched[t]
            n = P * kt
            kidx_parts.append(_wrap_idx(inv_k[off_k:off_k + n].reshape(P, kt), kt))
            vidx_parts.append(_wrap_idx(inv_v[off_v:off_v + n].reshape(P, kt), kt))
            off_k += n
            off_v += n
        in_maps.append({
            "kidx": np.ascontiguousarray(np.concatenate(kidx_parts, axis=1)),
            "vidx": np.ascontiguousarray(np.concatenate(vidx_parts, axis=1)),
            "bias": np.ascontiguousarray(bias[ids]),
            "query": np.ascontiguousarray(query[ids]),
            "key_table": ktab_c,
            "value_table": vtab_c,
        })
    return in_maps, core_ids_order, k_sched


def kernel(keys, values, pair_length, query, key_table, value_table):
    in_maps, core_ids_order, k_sched = _prep_inputs(
        keys, values, pair_length, query, key_table, value_table)
    nc = _get_program(k_sched)
    res = bass_utils.run_bass_kernel_spmd(nc, in_maps, core_ids=list(range(N_CORES)))
    out = np.empty((B, D), dtype=np.float32)
    for c in range(N_CORES):
        out[core_ids_order[c]] = res.results[c]["out"]
    return out


# revision 10
# speedup vs baseline: 1.1828x; 1.0182x over previous
"""KVMemory kernel for Trainium2 (8 NeuronCores, Bass/Tile).

Strategy (v17):
  - Data-parallel over batch. The host sorts the 4096 examples by
    pair_length, deals 128-example blocks snake-wise to the 8 cores, and
    tile slot j computes only k_sched[j] <= 50 keys (pair_length averages
    ~25 of 50, so half the key/value positions are softmax-masked; the
    sort turns that waste into real savings). Outputs are inverse-permuted
    on the host.
  - Embedding rows are fetched with the ANT dma_gather instruction in
    1024-row chunks (the SWDGE descriptor ring holds 1024 descriptors; a
    single larger gather wedges the Q7 ucode -- found the hard way).
    That is still ~50x fewer SWDGE instructions than per-column indirect
    DMA, whose multi-column offset form returns wrong data on hardware.
    dma_gather's int16 index limit is satisfied by host-side per-core
    table compaction: each core touches <= 25600 < 2^15 unique rows per
    table, so the host dedups (np.unique) and ships a compact fp16 table
    plus pre-wrapped int16 index tensors.
  - Three-engine compute split per tile (all ops verified against the
    neuronxcc ISA check and hardware):
      DVE:  broadcast-TT q*kemb product (2x perf mode), tensor_scalar 4x
            rowsums and scales, 2x tree adds, softmax small ops
      ACT:  activation(Copy) rowsums + wsum scales, Exp
      Pool: gather emission, wsum tensor_scalar+tensor_tensor ping-pong
  - Software pipelining: gathers for tile t+1 and the weighted sum of
    tile t-1 interleave with tile t's logits/softmax, ordered so no
    in-order engine queue blocks on unready work.
"""

import sys

if "/opt/trn_rl_repo" not in sys.path:
    sys.path.insert(0, "/opt/trn_rl_repo")

import numpy as np

import concourse.bass as bass
import concourse.mybir as mybir
import concourse.tile as tile
from concourse import bacc
from concourse import library_config
import concourse.bass_utils as bass_utils

N_CORES = 8
B = 4096
K = 50
D = 256
NUM_KEYS = 100000
NUM_VALUES = 100000
BC = B // N_CORES
P = 128
NTILES = BC // P
NBLOCKS = B // P
CAP = BC * K            # compact table rows per core (worst case, < 2^15)
MASK_NEG = np.float32(-1e30)

R_ACT = 0.42    # fraction of rowsums on ACT (rest on DVE)
W_ACT = 0.31    # fraction of wsum scales on ACT (tree on DVE)
W_POOL = 0.0    # base Pool wsum fraction (per-slot override below)
W_POOL_SLOT = [0.10, 0.20, 0.15, 0.0]  # Pool emits finish early; late tiles borrow it

_PROGRAM_CACHE = {}


def _splits(kt, slot=None):
    """Returns (rd, rc, wd, wc, wp): rowsum DVE/ACT counts; wsum DVE/ACT/Pool."""
    rc = int(round(kt * R_ACT))
    rd = kt - rc
    wfrac = W_POOL if slot is None else W_POOL_SLOT[slot]
    wp = int(round(kt * wfrac))
    wc = int(round(kt * W_ACT))
    wd = kt - wp - wc
    if wd < 1:
        wd = 1
        wc = max(0, kt - wd - wp)
        wp = kt - wd - wc
    return rd, rc, wd, wc, wp

GCHUNK = 8  # slabs per dma_gather (1024 rows; SWDGE ring holds 1024 descs)


def _build_program(k_sched):
    f16 = mybir.dt.float16
    f32 = mybir.dt.float32
    i16 = mybir.dt.int16
    A = mybir.AluOpType
    KMAX = max(k_sched)
    ksum = sum(k_sched)
    nc = bacc.Bacc("TRN2", target_bir_lowering=False, debug=False, num_devices=N_CORES)

    kidx_d = nc.dram_tensor("kidx", [P, ksum * 8], i16, kind="ExternalInput")
    vidx_d = nc.dram_tensor("vidx", [P, ksum * 8], i16, kind="ExternalInput")
    bias_d = nc.dram_tensor("bias", [BC, K], f32, kind="ExternalInput")
    query_d = nc.dram_tensor("query", [BC, D], f16, kind="ExternalInput")
    ktab_d = nc.dram_tensor("key_table", [CAP, D], f16, kind="ExternalInput")
    vtab_d = nc.dram_tensor("value_table", [CAP, D], f16, kind="ExternalInput")
    out_d = nc.dram_tensor("out", [BC, D], f32, kind="ExternalOutput")

    idx_off = [0]
    for kt in k_sched:
        idx_off.append(idx_off[-1] + kt * 8)

    with tile.TileContext(nc) as tc:
        with (
            tc.tile_pool(name="kpool", bufs=2) as kpool,
            tc.tile_pool(name="vpool", bufs=3) as vpool,
            tc.tile_pool(name="small", bufs=3) as sp,
        ):
            S = {}
            nc.gpsimd.load_library(library_config.mlp)

            def loads(t):
                kt = k_sched[t]
                rows = slice(t * P, (t + 1) * P)
                cols = slice(idx_off[t], idx_off[t + 1])
                kidx_t = sp.tile([P, KMAX * 8], i16, tag="kidx", name=f"kidx_{t}")
                nc.sync.dma_start(out=kidx_t[:, 0:kt * 8], in_=kidx_d[:, cols])
                vidx_t = sp.tile([P, KMAX * 8], i16, tag="vidx", name=f"vidx_{t}")
                nc.sync.dma_start(out=vidx_t[:, 0:kt * 8], in_=vidx_d[:, cols])
                bias_t = sp.tile([P, K], f32, tag="bias", name=f"bias_{t}")
                nc.sync.dma_start(out=bias_t[:, 0:kt], in_=bias_d[rows, 0:kt])
                q_t = sp.tile([P, D], f16, tag="q", name=f"q_{t}")
                nc.sync.dma_start(out=q_t[:], in_=query_d[rows, :])
                S[t] = dict(kidx=kidx_t, vidx=vidx_t, bias=bias_t, q=q_t)

            def _gather_chunks(dst, tab, idx, kt):
                for lo in range(0, kt, GCHUNK):
                    hi = min(lo + GCHUNK, kt)
                    n = P * (hi - lo)
                    nc.gpsimd.dma_gather(
                        dst[:, lo:hi, :], tab[:], idx[:, lo * 8:hi * 8], n, n, D,
                    )

            def gather_k(t):
                st = S[t]
                kt = k_sched[t]
                kemb = kpool.tile([P, KMAX, D], f16, tag="kemb", name=f"kemb_{t}")
                _gather_chunks(kemb, ktab_d, st["kidx"], kt)
                st["kemb"] = kemb

            def gather_v(t):
                st = S[t]
                kt = k_sched[t]
                vemb = vpool.tile([P, KMAX, D], f16, tag="vemb", name=f"vemb_{t}")
                _gather_chunks(vemb, vtab_d, st["vidx"], kt)
                st["vemb"] = vemb

            def act_wsum_copies(t):
                st = S[t]
                rd, rc, wd, wc, wp = _splits(k_sched[t], t)
                vemb, probs = st["vemb"], st["probs"]
                for k in range(wd, wd + wc):
                    nc.scalar.activation(
                        out=vemb[:, k, :], in_=vemb[:, k, :],
                        func=mybir.ActivationFunctionType.Copy,
                        bias=0.0, scale=probs[:, k:k + 1],
                    )

            def pool_wsum(t):
                st = S[t]
                kt = k_sched[t]
                rd, rc, wd, wc, wp = _splits(kt, t)
                if wp == 0:
                    st["accP"] = None
                    return
                vemb, probs = st["vemb"], st["probs"]
                base = wd + wc
                accP = sp.tile([P, D], f32, tag="accP", name=f"accP_{t}")
                scrP = sp.tile([P, D], f32, tag="scrP", name=f"scrP_{t}")
                nc.gpsimd.tensor_scalar(
                    out=accP[:], in0=vemb[:, base, :],
                    scalar1=probs[:, base:base + 1], scalar2=None, op0=A.mult,
                )
                for k in range(1, wp):
                    nc.gpsimd.tensor_scalar(
                        out=scrP[:], in0=vemb[:, base + k, :],
                        scalar1=probs[:, base + k:base + k + 1], scalar2=None,
                        op0=A.mult,
                    )
                    nc.gpsimd.tensor_tensor(
                        out=accP[:], in0=accP[:], in1=scrP[:], op=A.add,
                    )
                st["accP"] = accP

            def dve_prod_rowsums(t):
                st = S[t]
                kt = k_sched[t]
                rd, rc, wd, wc, wp = _splits(kt)
                kemb, q_t = st["kemb"], st["q"]
                logits = sp.tile([P, K], f32, tag="logits", name=f"logits_{t}")
                scrD = sp.tile([P, D], f16, tag="scrD", name=f"scrD_{t}")
                for lo in range(0, kt, GCHUNK):
                    hi = min(lo + GCHUNK, kt)
                    q_b = q_t[:].unsqueeze(1).broadcast_to([P, hi - lo, D])
                    nc.vector.tensor_tensor(
                        out=kemb[:, lo:hi, :], in0=kemb[:, lo:hi, :], in1=q_b,
                        op=A.mult,
                    )
                    for k in range(lo, min(hi, rd)):
                        nc.vector.tensor_scalar(
                            out=scrD[:], in0=kemb[:, k, :],
                            scalar1=1.0, scalar2=0.0, op0=A.mult, op1=A.add,
                            accum_out=logits[:, k:k + 1],
                        )
                st["logits"] = logits

            def act_rowsum_copies(t):
                st = S[t]
                kt = k_sched[t]
                rd, rc, wd, wc, wp = _splits(kt)
                kemb, logits = st["kemb"], st["logits"]
                scrA = sp.tile([P, D], f16, tag="scrA", name=f"scrA_{t}")
                for k in range(rd, kt):
                    nc.scalar.activation(
                        out=scrA[:], in_=kemb[:, k, :],
                        func=mybir.ActivationFunctionType.Copy,
                        bias=0.0, scale=1.0,
                        accum_out=logits[:, k:k + 1],
                    )

            def dve_bias_negmax(t):
                st = S[t]
                kt = k_sched[t]
                logits, bias_t = st["logits"], st["bias"]
                nc.vector.tensor_tensor(
                    out=logits[:, 0:kt], in0=logits[:, 0:kt], in1=bias_t[:, 0:kt],
                    op=A.add,
                )
                negmax = sp.tile([P, 1], f32, tag="negmax", name=f"negmax_{t}")
                nc.vector.tensor_reduce(
                    out=negmax[:], in_=logits[:, 0:kt], axis=mybir.AxisListType.X,
                    op=A.max, negate=True,
                )
                st["negmax"] = negmax

            def act_exp(t):
                st = S[t]
                kt = k_sched[t]
                probs = sp.tile([P, K], f32, tag="probs", name=f"probs_{t}")
                sumexp = sp.tile([P, 1], f32, tag="sumexp", name=f"sumexp_{t}")
                nc.scalar.activation(
                    out=probs[:, 0:kt], in_=st["logits"][:, 0:kt],
                    func=mybir.ActivationFunctionType.Exp,
                    bias=st["negmax"][:, :1], scale=1.0, accum_out=sumexp[:],
                )
                st["probs"] = probs
                st["sumexp"] = sumexp

            def dve_recip(t):
                st = S[t]
                inv = sp.tile([P, 1], f32, tag="inv", name=f"inv_{t}")
                nc.vector.reciprocal(out=inv[:], in_=st["sumexp"][:])
                st["inv"] = inv

            def dve_wsum(t):
                st = S[t]
                kt = k_sched[t]
                rd, rc, wd, wc, wp = _splits(kt, t)
                vemb, probs = st["vemb"], st["probs"]
                for k in range(wd):
                    nc.vector.tensor_scalar(
                        out=vemb[:, k, :], in0=vemb[:, k, :],
                        scalar1=probs[:, k:k + 1], scalar2=None, op0=A.mult,
                    )
                n = wd + wc
                while n > 1:
                    h = n // 2
                    nc.vector.tensor_tensor(
                        out=vemb[:, 0:h, :], in0=vemb[:, 0:h, :],
                        in1=vemb[:, n - h:n, :], op=A.add,
                    )
                    n = n - h

            def combine_store(t):
                st = S[t]
                out_t = sp.tile([P, D], f32, tag="out", name=f"out_{t}")
                inv = st["inv"]
                if st["accP"] is None:
                    nc.vector.tensor_scalar(
                        out=out_t[:], in0=st["vemb"][:, 0, :],
                        scalar1=inv[:, :1], scalar2=None, op0=A.mult,
                    )
                else:
                    nc.vector.tensor_tensor(
                        out=out_t[:], in0=st["vemb"][:, 0, :], in1=st["accP"][:],
                        op=A.add,
                    )
                    nc.vector.tensor_scalar(
                        out=out_t[:], in0=out_t[:],
                        scalar1=inv[:, :1], scalar2=None, op0=A.mult,
                    )
                rows = slice(t * P, (t + 1) * P)
                nc.sync.dma_start(out=out_d[rows, :], in_=out_t[:])

            loads(0)
            loads(1)
            gather_k(0)
            for t in range(NTILES):
                if t + 2 < NTILES:
                    loads(t + 2)
                if t + 1 < NTILES:
                    gather_k(t + 1)
                gather_v(t)
                if t - 1 >= 0:
                    act_wsum_copies(t - 1)
                    pool_wsum(t - 1)
                dve_prod_rowsums(t)
                act_rowsum_copies(t)
                dve_bias_negmax(t)
                if t - 1 >= 0:
                    dve_wsum(t - 1)
                act_exp(t)
                dve_recip(t)
                if t - 1 >= 0:
                    combine_store(t - 1)
            t = NTILES - 1
            act_wsum_copies(t)
            pool_wsum(t)
            dve_wsum(t)
            combine_store(t)

    nc.compile()
    return nc


def _get_program(k_sched):
    k_sched = tuple(k_sched)
    if k_sched not in _PROGRAM_CACHE:
        _PROGRAM_CACHE[k_sched] = _build_program(k_sched)
    return _PROGRAM_CACHE[k_sched]


SLOT_ORDER = [0, 2, 3, 1]


def _plan_sort(pair_length):
    order = np.argsort(pair_length, kind="stable")
    blocks = order.reshape(NBLOCKS, P)
    perm = [[None] * NTILES for _ in range(N_CORES)]
    k_raw = [0] * NTILES
    for j in range(NTILES):
        base = SLOT_ORDER[j] * N_CORES
        for c in range(N_CORES):
            bi = base + (c if j % 2 == 0 else N_CORES - 1 - c)
            perm[c][j] = blocks[bi]
            k_raw[j] = max(k_raw[j], int(pair_length[blocks[bi]].max()))
    k_sched = [min(K, max(4, k)) for k in k_raw]
    return perm, k_sched


def _wrap_idx(idx_pk, kt):
    """idx_pk: [P, kt] int -> wrapped int16 [P, kt*8].

    dma_gather consumes index i = tile[i % 16, i // 16] with list position
    i = k*P + p landing at out[p, k, :]; the 16-partition block is
    replicated to all 128 partitions.
    """
    lst = idx_pk.T.reshape(-1)                       # i = k*P + p
    wrapped = lst.reshape(-1, 16).T.astype(np.int16)  # [16, kt*8]
    return np.tile(wrapped, (P // 16, 1))


def _prep_inputs(keys, values, pair_length, query, key_table, value_table):
    keys = np.asarray(keys).astype(np.int32)
    values = np.asarray(values).astype(np.int32)
    pair_length = np.asarray(pair_length).astype(np.int32)
    query = np.asarray(query, dtype=np.float32).astype(np.float16)
    key_table = np.asarray(key_table, dtype=np.float32).astype(np.float16)
    value_table = np.asarray(value_table, dtype=np.float32).astype(np.float16)

    bias = np.where(np.arange(K, dtype=np.int32)[None, :] < pair_length[:, None],
                    np.float32(0.0), MASK_NEG).astype(np.float32)

    perm, k_sched = _plan_sort(pair_length)
    in_maps = []
    core_ids_order = []
    for c in range(N_CORES):
        ids = np.concatenate(perm[c])
        core_ids_order.append(ids)
        keys_c = keys[ids]
        values_c = values[ids]

        used_k = [keys_c[t * P:(t + 1) * P, 0:k_sched[t]] for t in range(NTILES)]
        used_v = [values_c[t * P:(t + 1) * P, 0:k_sched[t]] for t in range(NTILES)]
        uniq_k, inv_k = np.unique(np.concatenate([u.ravel() for u in used_k]),
                                  return_inverse=True)
        uniq_v, inv_v = np.unique(np.concatenate([u.ravel() for u in used_v]),
                                  return_inverse=True)
        ktab_c = np.zeros((CAP, D), dtype=np.float16)
        ktab_c[:len(uniq_k)] = key_table[uniq_k]
        vtab_c = np.zeros((CAP, D), dtype=np.float16)
        vtab_c[:len(uniq_v)] = value_table[uniq_v]

        kidx_parts, vidx_parts = [], []
        off_k = off_v = 0
        for t in range(NTILES):
            kt = k_sched[t]
            n = P * kt
            kidx_parts.append(_wrap_idx(inv_k[off_k:off_k + n].reshape(P, kt), kt))
            vidx_parts.append(_wrap_idx(inv_v[off_v:off_v + n].reshape(P, kt), kt))
            off_k += n
            off_v += n
        in_maps.append({
            "kidx": np.ascontiguousarray(np.concatenate(kidx_parts, axis=1)),
            "vidx": np.ascontiguousarray(np.concatenate(vidx_parts, axis=1)),
            "bias": np.ascontiguousarray(bias[ids]),
            "query": np.ascontiguousarray(query[ids]),
            "key_table": ktab_c,
            "value_table": vtab_c,
        })
    return in_maps, core_ids_order, k_sched


def kernel(keys, values, pair_length, query, key_table, value_table):
    in_maps, core_ids_order, k_sched = _prep_inputs(
        keys, values, pair_length, query, key_table, value_table)
    nc = _get_program(k_sched)
    res = bass_utils.run_bass_kernel_spmd(nc, in_maps, core_ids=list(range(N_CORES)))
    out = np.empty((B, D), dtype=np.float32)
    for c in range(N_CORES):
        out[core_ids_order[c]] = res.results[c]["out"]
    return out


# revision 11
# speedup vs baseline: 1.1885x; 1.0048x over previous
"""KVMemory kernel for Trainium2 (8 NeuronCores, Bass/Tile).

Strategy (v20):
  - Data-parallel over batch. The host sorts the 4096 examples by
    pair_length, deals 128-example blocks snake-wise to the 8 cores, and
    tile slot j computes only k_sched[j] <= 50 keys (pair_length averages
    ~25 of 50, so half the key/value positions are softmax-masked; the
    sort turns that waste into real savings). Outputs are inverse-permuted
    on the host.
  - Embedding rows are fetched with the ANT dma_gather instruction in
    1024-row chunks (the SWDGE descriptor ring holds 1024 descriptors; a
    single larger gather wedges the Q7 ucode -- found the hard way).
    That is still ~50x fewer SWDGE instructions than per-column indirect
    DMA, whose multi-column offset form returns wrong data on hardware.
    dma_gather's int16 index limit is satisfied by host-side per-core
    table compaction: each core touches <= 25600 < 2^15 unique rows per
    table, so the host dedups (np.unique) and ships a compact fp16 table
    plus pre-wrapped int16 index tensors.
  - Three-engine compute split per tile (all ops verified against the
    neuronxcc ISA check and hardware):
      DVE:  broadcast-TT q*kemb product (2x perf mode), tensor_scalar 4x
            rowsums and scales, 2x tree adds, softmax small ops
      ACT:  activation(Copy) rowsums + wsum scales, Exp
      Pool: gather emission, wsum tensor_scalar+tensor_tensor ping-pong
  - Software pipelining: gathers for tile t+1 and the weighted sum of
    tile t-1 interleave with tile t's logits/softmax, ordered so no
    in-order engine queue blocks on unready work.
"""

import sys

if "/opt/trn_rl_repo" not in sys.path:
    sys.path.insert(0, "/opt/trn_rl_repo")

import numpy as np

import concourse.bass as bass
import concourse.mybir as mybir
import concourse.tile as tile
from concourse import bacc
from concourse import library_config
import concourse.bass_utils as bass_utils

N_CORES = 8
B = 4096
K = 50
D = 256
NUM_KEYS = 100000
NUM_VALUES = 100000
BC = B // N_CORES
P = 128
NTILES = BC // P
NBLOCKS = B // P
CAP = BC * K            # compact table rows per core (worst case, < 2^15)
MASK_NEG = np.float32(-1e30)

R_ACT = 0.42    # fraction of rowsums on ACT (rest on DVE)
W_ACT = 0.31    # fraction of wsum scales on ACT (tree on DVE)
W_POOL = 0.0    # base Pool wsum fraction (per-slot override below)
W_POOL_SLOT = [0.10, 0.20, 0.15, 0.0]  # Pool emits finish early; late tiles borrow it

_PROGRAM_CACHE = {}


def _splits(kt, slot=None):
    """Returns (rd, rc, wd, wc, wp): rowsum DVE/ACT counts; wsum DVE/ACT/Pool."""
    rc = int(round(kt * R_ACT))
    rd = kt - rc
    wfrac = W_POOL if slot is None else W_POOL_SLOT[slot]
    wp = int(round(kt * wfrac))
    wc = int(round(kt * W_ACT))
    wd = kt - wp - wc
    if wd < 1:
        wd = 1
        wc = max(0, kt - wd - wp)
        wp = kt - wd - wc
    return rd, rc, wd, wc, wp

GCHUNK = 8  # slabs per dma_gather (1024 rows; SWDGE ring holds 1024 descs)


def _build_program(k_sched):
    f16 = mybir.dt.float16
    f32 = mybir.dt.float32
    i16 = mybir.dt.int16
    A = mybir.AluOpType
    KMAX = max(k_sched)
    ksum = sum(k_sched)
    nc = bacc.Bacc("TRN2", target_bir_lowering=False, debug=False, num_devices=N_CORES)

    kvidx_d = nc.dram_tensor("kvidx", [P, 2 * ksum * 8], i16, kind="ExternalInput")
    qbias_d = nc.dram_tensor("qbias", [BC, D + K], f16, kind="ExternalInput")
    ktab_d = nc.dram_tensor("key_table", [CAP, D], f16, kind="ExternalInput")
    vtab_d = nc.dram_tensor("value_table", [CAP, D], f16, kind="ExternalInput")
    out_d = nc.dram_tensor("out", [BC, D], f16, kind="ExternalOutput")

    idx_off = [0]
    for kt in k_sched:
        idx_off.append(idx_off[-1] + kt * 8)

    with tile.TileContext(nc) as tc:
        with (
            tc.tile_pool(name="kpool", bufs=2) as kpool,
            tc.tile_pool(name="vpool", bufs=3) as vpool,
            tc.tile_pool(name="small", bufs=3) as sp,
        ):
            S = {}
            nc.gpsimd.load_library(library_config.mlp)

            def loads(t):
                kt = k_sched[t]
                rows = slice(t * P, (t + 1) * P)
                kvcols = slice(2 * idx_off[t], 2 * idx_off[t + 1])
                kvidx_t = sp.tile([P, 2 * KMAX * 8], i16, tag="kvidx",
                                  name=f"kvidx_{t}")
                nc.sync.dma_start(out=kvidx_t[:, 0:2 * kt * 8], in_=kvidx_d[:, kvcols])
                qbias_t = sp.tile([P, D + K], f16, tag="qbias", name=f"qbias_{t}")
                nc.sync.dma_start(out=qbias_t[:, 0:D + kt], in_=qbias_d[rows, 0:D + kt])
                S[t] = dict(
                    kidx=kvidx_t[:, 0:kt * 8],
                    vidx=kvidx_t[:, kt * 8:2 * kt * 8],
                    bias=qbias_t[:, D:D + kt],
                    q=qbias_t[:, 0:D],
                )

            def _gather_chunks(dst, tab, idx, kt):
                for lo in range(0, kt, GCHUNK):
                    hi = min(lo + GCHUNK, kt)
                    n = P * (hi - lo)
                    nc.gpsimd.dma_gather(
                        dst[:, lo:hi, :], tab[:], idx[:, lo * 8:hi * 8], n, n, D,
                    )

            def gather_k(t):
                st = S[t]
                kt = k_sched[t]
                kemb = kpool.tile([P, KMAX, D], f16, tag="kemb", name=f"kemb_{t}")
                _gather_chunks(kemb, ktab_d, st["kidx"], kt)
                st["kemb"] = kemb

            def gather_v(t):
                st = S[t]
                kt = k_sched[t]
                vemb = vpool.tile([P, KMAX, D], f16, tag="vemb", name=f"vemb_{t}")
                _gather_chunks(vemb, vtab_d, st["vidx"], kt)
                st["vemb"] = vemb

            def act_wsum_copies(t):
                st = S[t]
                rd, rc, wd, wc, wp = _splits(k_sched[t], t)
                vemb, probs = st["vemb"], st["probs"]
                for k in range(wd, wd + wc):
                    nc.scalar.activation(
                        out=vemb[:, k, :], in_=vemb[:, k, :],
                        func=mybir.ActivationFunctionType.Copy,
                        bias=0.0, scale=probs[:, k:k + 1],
                    )

            def pool_wsum(t):
                st = S[t]
                kt = k_sched[t]
                rd, rc, wd, wc, wp = _splits(kt, t)
                if wp == 0:
                    st["accP"] = None
                    return
                vemb, probs = st["vemb"], st["probs"]
                base = wd + wc
                accP = sp.tile([P, D], f32, tag="accP", name=f"accP_{t}")
                scrP = sp.tile([P, D], f32, tag="scrP", name=f"scrP_{t}")
                nc.gpsimd.tensor_scalar(
                    out=accP[:], in0=vemb[:, base, :],
                    scalar1=probs[:, base:base + 1], scalar2=None, op0=A.mult,
                )
                for k in range(1, wp):
                    nc.gpsimd.tensor_scalar(
                        out=scrP[:], in0=vemb[:, base + k, :],
                        scalar1=probs[:, base + k:base + k + 1], scalar2=None,
                        op0=A.mult,
                    )
                    nc.gpsimd.tensor_tensor(
                        out=accP[:], in0=accP[:], in1=scrP[:], op=A.add,
                    )
                st["accP"] = accP

            def dve_prod_rowsums(t):
                st = S[t]
                kt = k_sched[t]
                rd, rc, wd, wc, wp = _splits(kt)
                kemb, q_t = st["kemb"], st["q"]
                logits = sp.tile([P, K], f32, tag="logits", name=f"logits_{t}")
                scrD = sp.tile([P, D], f16, tag="scrD", name=f"scrD_{t}")
                for lo in range(0, kt, GCHUNK):
                    hi = min(lo + GCHUNK, kt)
                    q_b = q_t[:].unsqueeze(1).broadcast_to([P, hi - lo, D])
                    nc.vector.tensor_tensor(
                        out=kemb[:, lo:hi, :], in0=kemb[:, lo:hi, :], in1=q_b,
                        op=A.mult,
                    )
                    for k in range(lo, min(hi, rd)):
                        nc.vector.tensor_scalar(
                            out=scrD[:], in0=kemb[:, k, :],
                            scalar1=1.0, scalar2=0.0, op0=A.mult, op1=A.add,
                            accum_out=logits[:, k:k + 1],
                        )
                st["logits"] = logits

            def act_rowsum_copies(t):
                st = S[t]
                kt = k_sched[t]
                rd, rc, wd, wc, wp = _splits(kt)
                kemb, logits = st["kemb"], st["logits"]
                scrA = sp.tile([P, D], f16, tag="scrA", name=f"scrA_{t}")
                for k in range(rd, kt):
                    nc.scalar.activation(
                        out=scrA[:], in_=kemb[:, k, :],
                        func=mybir.ActivationFunctionType.Copy,
                        bias=0.0, scale=1.0,
                        accum_out=logits[:, k:k + 1],
                    )

            def dve_bias_negmax(t):
                st = S[t]
                kt = k_sched[t]
                logits, bias_t = st["logits"], st["bias"]
                nc.vector.tensor_tensor(
                    out=logits[:, 0:kt], in0=logits[:, 0:kt], in1=bias_t,
                    op=A.add,
                )
                negmax = sp.tile([P, 1], f32, tag="negmax", name=f"negmax_{t}")
                nc.vector.tensor_reduce(
                    out=negmax[:], in_=logits[:, 0:kt], axis=mybir.AxisListType.X,
                    op=A.max, negate=True,
                )
                st["negmax"] = negmax

            def act_exp(t):
                st = S[t]
                kt = k_sched[t]
                probs = sp.tile([P, K], f32, tag="probs", name=f"probs_{t}")
                sumexp = sp.tile([P, 1], f32, tag="sumexp", name=f"sumexp_{t}")
                nc.scalar.activation(
                    out=probs[:, 0:kt], in_=st["logits"][:, 0:kt],
                    func=mybir.ActivationFunctionType.Exp,
                    bias=st["negmax"][:, :1], scale=1.0, accum_out=sumexp[:],
                )
                st["probs"] = probs
                st["sumexp"] = sumexp

            def dve_recip(t):
                st = S[t]
                inv = sp.tile([P, 1], f32, tag="inv", name=f"inv_{t}")
                nc.vector.reciprocal(out=inv[:], in_=st["sumexp"][:])
                st["inv"] = inv

            def dve_wsum(t):
                st = S[t]
                kt = k_sched[t]
                rd, rc, wd, wc, wp = _splits(kt, t)
                vemb, probs = st["vemb"], st["probs"]
                for k in range(wd):
                    nc.vector.tensor_scalar(
                        out=vemb[:, k, :], in0=vemb[:, k, :],
                        scalar1=probs[:, k:k + 1], scalar2=None, op0=A.mult,
                    )
                n = wd + wc
                while n > 1:
                    h = n // 2
                    nc.vector.tensor_tensor(
                        out=vemb[:, 0:h, :], in0=vemb[:, 0:h, :],
                        in1=vemb[:, n - h:n, :], op=A.add,
                    )
                    n = n - h

            def combine_store(t):
                st = S[t]
                out_t = sp.tile([P, D], f16, tag="out", name=f"out_{t}")
                inv = st["inv"]
                if st["accP"] is None:
                    nc.vector.tensor_scalar(
                        out=out_t[:], in0=st["vemb"][:, 0, :],
                        scalar1=inv[:, :1], scalar2=None, op0=A.mult,
                    )
                else:
                    nc.vector.tensor_tensor(
                        out=out_t[:], in0=st["vemb"][:, 0, :], in1=st["accP"][:],
                        op=A.add,
                    )
                    nc.vector.tensor_scalar(
                        out=out_t[:], in0=out_t[:],
                        scalar1=inv[:, :1], scalar2=None, op0=A.mult,
                    )
                rows = slice(t * P, (t + 1) * P)
                nc.sync.dma_start(out=out_d[rows, :], in_=out_t[:])

            loads(0)
            loads(1)
            gather_k(0)
            for t in range(NTILES):
                if t + 2 < NTILES:
                    loads(t + 2)
                if t + 1 < NTILES:
                    gather_k(t + 1)
                gather_v(t)
                if t - 1 >= 0:
                    act_wsum_copies(t - 1)
                    pool_wsum(t - 1)
                dve_prod_rowsums(t)
                act_rowsum_copies(t)
                dve_bias_negmax(t)
                if t - 1 >= 0:
                    dve_wsum(t - 1)
                act_exp(t)
                dve_recip(t)
                if t - 1 >= 0:
                    combine_store(t - 1)
            t = NTILES - 1
            act_wsum_copies(t)
            pool_wsum(t)
            dve_wsum(t)
            combine_store(t)

    nc.compile()
    return nc


def _get_program(k_sched):
    k_sched = tuple(k_sched)
    if k_sched not in _PROGRAM_CACHE:
        _PROGRAM_CACHE[k_sched] = _build_program(k_sched)
    return _PROGRAM_CACHE[k_sched]


SLOT_ORDER = [0, 2, 3, 1]


def _plan_sort(pair_length):
    order = np.argsort(pair_length, kind="stable")
    blocks = order.reshape(NBLOCKS, P)
    perm = [[None] * NTILES for _ in range(N_CORES)]
    k_raw = [0] * NTILES
    for j in range(NTILES):
        base = SLOT_ORDER[j] * N_CORES
        for c in range(N_CORES):
            bi = base + (c if j % 2 == 0 else N_CORES - 1 - c)
            perm[c][j] = blocks[bi]
            k_raw[j] = max(k_raw[j], int(pair_length[blocks[bi]].max()))
    k_sched = [min(K, max(4, k)) for k in k_raw]
    return perm, k_sched


def _wrap_idx(idx_pk, kt):
    """idx_pk: [P, kt] int -> wrapped int16 [P, kt*8].

    dma_gather consumes index i = tile[i % 16, i // 16] with list position
    i = k*P + p landing at out[p, k, :]; the 16-partition block is
    replicated to all 128 partitions.
    """
    lst = idx_pk.T.reshape(-1)                       # i = k*P + p
    wrapped = lst.reshape(-1, 16).T.astype(np.int16)  # [16, kt*8]
    return np.tile(wrapped, (P // 16, 1))


def _prep_inputs(keys, values, pair_length, query, key_table, value_table):
    keys = np.asarray(keys).astype(np.int32)
    values = np.asarray(values).astype(np.int32)
    pair_length = np.asarray(pair_length).astype(np.int32)
    query = np.asarray(query, dtype=np.float32).astype(np.float16)
    key_table = np.asarray(key_table, dtype=np.float32).astype(np.float16)
    value_table = np.asarray(value_table, dtype=np.float32).astype(np.float16)

    bias = np.where(np.arange(K, dtype=np.int32)[None, :] < pair_length[:, None],
                    np.float32(0.0), np.float32(-60000.0)).astype(np.float32)

    perm, k_sched = _plan_sort(pair_length)
    in_maps = []
    core_ids_order = []
    for c in range(N_CORES):
        ids = np.concatenate(perm[c])
        core_ids_order.append(ids)
        keys_c = keys[ids]
        values_c = values[ids]

        used_k = [keys_c[t * P:(t + 1) * P, 0:k_sched[t]] for t in range(NTILES)]
        used_v = [values_c[t * P:(t + 1) * P, 0:k_sched[t]] for t in range(NTILES)]
        uniq_k, inv_k = np.unique(np.concatenate([u.ravel() for u in used_k]),
                                  return_inverse=True)
        uniq_v, inv_v = np.unique(np.concatenate([u.ravel() for u in used_v]),
                                  return_inverse=True)
        ktab_c = np.zeros((CAP, D), dtype=np.float16)
        ktab_c[:len(uniq_k)] = key_table[uniq_k]
        vtab_c = np.zeros((CAP, D), dtype=np.float16)
        vtab_c[:len(uniq_v)] = value_table[uniq_v]

        kidx_parts, vidx_parts = [], []
        off_k = off_v = 0
        for t in range(NTILES):
            kt = k_sched[t]
            n = P * kt
            kidx_parts.append(_wrap_idx(inv_k[off_k:off_k + n].reshape(P, kt), kt))
            vidx_parts.append(_wrap_idx(inv_v[off_v:off_v + n].reshape(P, kt), kt))
            off_k += n
            off_v += n
        kv_parts = []
        for kp_, vp_ in zip(kidx_parts, vidx_parts):
            kv_parts.append(kp_)
            kv_parts.append(vp_)
        qbias = np.concatenate([query[ids], bias[ids].astype(np.float16)], axis=1)
        in_maps.append({
            "kvidx": np.ascontiguousarray(np.concatenate(kv_parts, axis=1)),
            "qbias": np.ascontiguousarray(qbias),
            "key_table": ktab_c,
            "value_table": vtab_c,
        })
    return in_maps, core_ids_order, k_sched


def kernel(keys, values, pair_length, query, key_table, value_table):
    in_maps, core_ids_order, k_sched = _prep_inputs(
        keys, values, pair_length, query, key_table, value_table)
    nc = _get_program(k_sched)
    res = bass_utils.run_bass_kernel_spmd(nc, in_maps, core_ids=list(range(N_CORES)))
    out = np.empty((B, D), dtype=np.float32)
    for c in range(N_CORES):
        out[core_ids_order[c]] = res.results[c]["out"].astype(np.float32)
    return out
